# revision 8
# baseline (speedup 1.0000x reference)
"""Trainium2 Bass kernel for nn_DCWTv2InferenceCache (segment-tree cached attention).

Sharding: tensor-parallel over the 16-head axis -> 8 cores x 2 heads.
Each core streams its (50000, 2*64) f32 slice of the value cache from HBM in
2MB sub-chunks, reduces segment-tree nodes to (64, 128) slot-sums on the PE
(sliding one-hot selection weights, PSUM accumulation), then runs the per-node
depth-projected attention epilogue on-device. Output is head-sharded (2, 64)
per core, gathered on host. No cross-device communication.

Perf structure (vs the first working version):
- the 64 stage-A selection weights are sliding slices B[:, 128-r:192-r] of a
  single (128,192) buffer with one 1.0 per partition -> the old 2.1MB selb
  constant DMA is gone (96KB now).
- the token stream is issued as 2MB sub-chunk DMAs on the sync HWDGE ring
  (constants/prefetches ride the scalar ring), each with its own tile, so the
  PE chases the stream at 2MB granularity instead of 8MB supersteps; the last
  chunks are 1MB to shrink the end-of-stream compute tail.
- softmax scales (softplus temp, mean 64/L, 1/NT) are host-folded; the
  max-subtraction is dropped (logits are O(1) by construction), and the PSUM
  c-halves fold with one DVE add, shortening the tail epilogue chain.
"""

import math
import os
import sys

if "/opt/trn_rl_repo" not in sys.path:
    sys.path.insert(0, "/opt/trn_rl_repo")

import numpy as np

import concourse.bass as bass
import concourse.mybir as mybir
import concourse.tile as tile
from concourse import bacc
from concourse.bass_utils import run_bass_kernel_spmd

# --- problem constants (from the reference nn.Module) ---
MAX_LEN = 65536
NUM_HEADS = 16
HEAD_DIM = 64
K_MAX = 64
LOCAL_WINDOW = 512
LOG_N = 17
LEAF_START = 2**LOG_N

N_CORES = 8
HPC = NUM_HEADS // N_CORES        # heads per core = 2
F = HPC * HEAD_DIM                # feature width per core = 128
NTOK = 50000                      # v_tokens buffer length

CHUNK = 128                       # tokens per matmul tile (partition dim)
SUPER = 16384                     # tokens per stage-A superstep (2 PSUM c-slots)
STAGE_A_MODE = "r64"              # kept for test.py compat

SUBR = int(os.environ.get("DCWT_SUBR", "16"))     # r-slots per stream sub-chunk
TAILR = int(os.environ.get("DCWT_TAILR", "16"))    # r-slots of final sub-chunks
VBUFS = int(os.environ.get("DCWT_VBUFS", "8"))    # stream tile ring depth

SELB_W = 64 + 192                 # [sel(64) | slide buffer(192, one-hot at 128)]
SLIDE0 = 64


def _cblob_layout(NT):
    """Column offsets inside the packed (128, W) f32 constants blob."""
    nt = max(NT, 1)
    off = {}
    off["ident"] = 0
    off["qbd"] = 128
    off["qT"] = 130
    off["rs"] = 132            # (2, NT) folded softmax logit scales
    off["zmul"] = 132 + nt     # (2, NT) folded Z multipliers
    off["wTI"] = 132 + 2 * nt
    return off, 132 + 2 * nt + nt * 64


f32 = mybir.dt.float32
f32r = mybir.dt.float32r
AF = mybir.ActivationFunctionType
AX = mybir.AxisListType

_last_results = None  # stash for test harness introspection


def cover_set(pos):
    """O(log n) segment-tree nodes covering prefix [0..pos-1]: (start, L, depth),
    ascending start (binary decomposition of pos)."""
    if pos <= 0:
        return []
    l, r = LEAF_START, LEAF_START + min(pos, MAX_LEN)
    out = []
    while l < r:
        if l & 1:
            d = LOG_N - int(math.floor(math.log2(l)))
            out.append(((l << d) - LEAF_START, 1 << d, d))
            l += 1
        if r & 1:
            r -= 1
            d = LOG_N - int(math.floor(math.log2(r)))
            out.append(((r << d) - LEAF_START, 1 << d, d))
        l >>= 1
        r >>= 1
    return sorted(out)


def _build_program(pos, mode="r64"):
    """Build the single-core Bass/Tile program (same program for all 8 cores)."""
    nodes = cover_set(pos)
    big = [(s, L, d) for (s, L, d) in nodes if L > K_MAX]
    small = [(s, L, d) for (s, L, d) in nodes if L <= K_MAX]
    tree = big + small                                          # epilogue order
    NT = len(tree)
    stream = [i for i in range(len(big)) if big[i][1] >= SUPER]  # r64 path
    oldp = [i for i in range(len(big)) if big[i][1] < SUPER]     # prefetch path
    n_loc = min(pos, LOCAL_WINDOW)
    assert n_loc % CHUNK == 0, "local window must be chunk-aligned for this build"
    NLC = n_loc // CHUNK

    inv_sqrt_d = 1.0 / math.sqrt(HEAD_DIM)

    nc = bacc.Bacc("TRN2", target_bir_lowering=False, debug=False)

    v = nc.dram_tensor("v", [NTOK, F], f32, kind="ExternalInput")
    CBOFF, CB_W = _cblob_layout(NT)
    CB_IDENT, CB_QBD, CB_QT = CBOFF["ident"], CBOFF["qbd"], CBOFF["qT"]
    CB_RS, CB_ZMUL, CB_WTI = CBOFF["rs"], CBOFF["zmul"], CBOFF["wTI"]
    selb_d = nc.dram_tensor("selb", [CHUNK, SELB_W], f32r, kind="ExternalInput")
    cblob_d = nc.dram_tensor("cblob", [CHUNK, CB_W], f32, kind="ExternalInput")
    o = nc.dram_tensor("o", [HPC, HEAD_DIM], f32, kind="ExternalOutput")

    with tile.TileContext(nc) as tc:
        with (
            tc.tile_pool(name="consts", bufs=1) as cpool,
            tc.tile_pool(name="vstream", bufs=VBUFS) as vpool,
            tc.tile_pool(name="fsb", bufs=2) as fpool,
            tc.tile_pool(name="ep_sb", bufs=2) as spool,
            tc.tile_pool(name="xsb", bufs=3) as xpool,
            tc.tile_pool(name="acc_ps", bufs=2, space=bass.MemorySpace.PSUM) as apool,
            tc.tile_pool(name="ep_ps", bufs=1, space=bass.MemorySpace.PSUM) as eppool,
            tc.tile_pool(name="out_ps", bufs=1, space=bass.MemorySpace.PSUM) as opool,
        ):
            # ---- constants + prefetches ride the scalar HWDGE ring; the sync
            # ring carries ONLY the big token stream so it starts immediately.
            cb = cpool.tile([CHUNK, CB_W], f32)
            nc.scalar.dma_start(cb[:], cblob_d[:])
            ident_sb = cb[:, CB_IDENT : CB_IDENT + CHUNK]
            qbd_sb = cb[:, CB_QBD : CB_QBD + HPC]
            qT_sb = cb[0:HEAD_DIM, CB_QT : CB_QT + HPC]
            rs_sb = cb[0:HPC, CB_RS : CB_RS + max(NT, 1)]
            zmul_sb = cb[0:HPC, CB_ZMUL : CB_ZMUL + max(NT, 1)]

            selb_t = cpool.tile([CHUNK, SELB_W], f32r)
            nc.scalar.dma_start(selb_t[:], selb_d[:])
            selb = selb_t[:]
            sel_sb = selb[:, 0:K_MAX]

            def slide_lhsT(r):
                # (128, 64) weight with column r all-ones
                return selb[:, SLIDE0 + 128 - r : SLIDE0 + 192 - r]

            def wTI_slice(n):
                return cb[0:HEAD_DIM, CB_WTI + n * HEAD_DIM : CB_WTI + (n + 1) * HEAD_DIM]

            # ---- prefetch raw tail data (small nodes + local window) ----
            small_tiles = []
            for si, (start_s, L_s, _d) in enumerate(small):
                fsm = cpool.tile([K_MAX, F], f32, name=f"fsm{si}", tag=f"fsm{si}")
                nc.scalar.dma_start(fsm[0:L_s, :], v[start_s : start_s + L_s, :])
                small_tiles.append(fsm)
            lstart = pos - n_loc
            fl_sb = cpool.tile([CHUNK, NLC, F], f32)
            nc.scalar.dma_start(
                fl_sb[:],
                v[lstart : lstart + n_loc, :].rearrange("(c p) f -> p c f", p=CHUNK),
            )

            # old-path (64 < L < SUPER) node prefetch, chunked layout
            oldpath_tiles = {}
            for i in oldp:
                start_b, L_b, _d = big[i]
                nch_b = L_b // CHUNK
                vo = cpool.tile(
                    [CHUNK, nch_b, F], f32r, name=f"vo{start_b}", tag=f"vo{start_b}"
                )
                srcb = v[start_b : start_b + L_b, :].bitcast(f32r)
                nc.scalar.dma_start(
                    vo[:], srcb.rearrange("(c p) f -> p c f", p=CHUNK)
                )
                oldpath_tiles[start_b] = vo

            # ---- all tree-node q_depth projections upfront (block-diag) ----
            qd_all = cpool.tile([2 * HEAD_DIM, max(NT, 1), HPC], f32)
            nc.vector.memset(qd_all[:], 0.0)
            for n in range(NT):
                qd_ps = eppool.tile([2 * HEAD_DIM, HPC], f32, tag="qd_ps")
                nc.tensor.matmul(
                    qd_ps[0:HEAD_DIM, 0:1],
                    wTI_slice(n), qT_sb[:, 0:1], start=True, stop=True,
                )
                nc.tensor.matmul(
                    qd_ps[HEAD_DIM : 2 * HEAD_DIM, 1:2],
                    wTI_slice(n), qT_sb[:, 1:2], start=True, stop=True,
                )
                nc.scalar.copy(qd_all[0:HEAD_DIM, n, 0:1], qd_ps[0:HEAD_DIM, 0:1])
                nc.scalar.copy(
                    qd_all[HEAD_DIM : 2 * HEAD_DIM, n, 1:2],
                    qd_ps[HEAD_DIM : 2 * HEAD_DIM, 1:2],
                )

            # ---- cross-node output accumulator (2, 128) PSUM ----
            out_ps = opool.tile([HPC, F], f32)
            n_out_mm = NT + NLC
            out_mm = [0]  # running count, for start/stop flags

            def out_matmul(wT_sb_ap, f_sb_ap):
                nc.tensor.matmul(
                    out_ps[:], wT_sb_ap, f_sb_ap,
                    start=(out_mm[0] == 0), stop=(out_mm[0] == n_out_mm - 1),
                )
                out_mm[0] += 1

            def softmax_weights(s_ps_ap, K, node_i):
                """exp(s*rs_n) / (Z * zmul_n): no max-subtraction (logits are
                O(1) by construction); rs_n folds softplus-temp, sqrt(D) and
                the 64/L mean; zmul_n folds NT and L/64."""
                ebd = xpool.tile([HPC, K], f32, tag="esb")
                zt = xpool.tile([HPC, 1], f32, tag="zt")
                if node_i >= 0:
                    nc.scalar.activation(
                        ebd[:], s_ps_ap, AF.Exp,
                        scale=rs_sb[:, node_i : node_i + 1], accum_out=zt[:],
                    )
                    zs = xpool.tile([HPC, 1], f32, tag="zs")
                    nc.vector.tensor_scalar_mul(
                        zs[:], zt[:], zmul_sb[:, node_i : node_i + 1]
                    )
                    zt = zs
                else:  # local window
                    nc.scalar.activation(
                        ebd[:], s_ps_ap, AF.Exp, scale=inv_sqrt_d, accum_out=zt[:],
                    )
                rz = xpool.tile([HPC, 1], f32, tag="rz")
                nc.vector.reciprocal(rz[:], zt[:])
                w_sb = xpool.tile([HPC, K], f32, tag="wsb")
                nc.vector.tensor_scalar_mul(w_sb[:], ebd[:], rz[:])
                return w_sb

            def tree_epilogue(node_i, f_sb_ap, K):
                """Attention of depth-projected query against f (K, F), one node."""
                fT_ps = eppool.tile([F, K_MAX], f32, tag="fT_ps", bufs=1)
                nc.tensor.transpose(fT_ps[:, 0:K], f_sb_ap, ident_sb[0:K, 0:K])
                fT_sb = spool.tile([F, K_MAX], f32, tag="fT_sb")
                nc.scalar.copy(fT_sb[:, 0:K], fT_ps[:, 0:K])
                s_ps = eppool.tile([HPC, K_MAX], f32, tag="s_ps", bufs=1)
                nc.tensor.matmul(
                    s_ps[:, 0:K], qd_all[:, node_i, :], fT_sb[:, 0:K],
                    start=True, stop=True,
                )
                w_sb = softmax_weights(s_ps[:, 0:K], K, node_i)
                wT_ps = eppool.tile([K_MAX, HPC], f32, tag="qd_ps")
                nc.tensor.transpose(wT_ps[0:K, :], w_sb[:], ident_sb[0:HPC, 0:HPC])
                wT_sb = spool.tile([K_MAX, HPC], f32, tag="wT_sb")
                nc.scalar.copy(wT_sb[0:K, :], wT_ps[0:K, :])
                out_matmul(wT_sb[0:K, :], f_sb_ap)

            def fold_halves(ps2, nhalves):
                """(64, 2, F) PSUM -> (64, F) SBUF slot-sums (scales folded)."""
                f_sb = fpool.tile([K_MAX, F], f32, tag="f")
                if nhalves > 1:
                    # walrus allows only one non-scalar PSUM input per op
                    ha = fpool.tile([K_MAX, F], f32, tag="ha")
                    nc.scalar.copy(ha[:], ps2[:, 0, :])
                    nc.vector.tensor_add(f_sb[:], ha[:], ps2[:, 1, :])
                else:
                    nc.scalar.copy(f_sb[:], ps2[:, 0, :])
                return f_sb

            # ================= streamed stage A (L >= SUPER nodes) ==========
            # Chunk list: (node_i, superstep_base, r0, r1, start?, stop?)
            def emit_stream_node(node_i, start, L, last_node):
                SS = L // SUPER
                ps2 = apool.tile([K_MAX, 2, F], f32, tag="acc")
                n_mm = SS * K_MAX
                done = 0
                for s in range(SS):
                    base = start + s * SUPER
                    src4 = (
                        v[base : base + SUPER, :]
                        .bitcast(f32r)
                        .rearrange("(c q r) f -> q c r f", c=2, q=CHUNK, r=K_MAX)
                    )
                    final_ss = last_node and s == SS - 1
                    groups = []
                    r0 = 0
                    while r0 < K_MAX:
                        rem = K_MAX - r0
                        if final_ss and rem <= 2 * TAILR:
                            w = TAILR if rem > TAILR else rem
                        else:
                            w = min(SUBR, rem)
                        groups.append((r0, r0 + w))
                        r0 += w
                    for (g0, g1) in groups:
                        gw = g1 - g0
                        vt = vpool.tile(
                            [CHUNK, 2, gw, F], f32r, tag=f"vb{gw}",
                            bufs=VBUFS if gw == SUBR else 2,
                        )
                        nc.sync.dma_start(vt[:], src4[:, :, g0:g1, :])
                        for rr in range(g0, g1):
                            nc.tensor.matmul(
                                ps2[:, :, :], slide_lhsT(rr), vt[:, :, rr - g0, :],
                                start=(done == 0), stop=(done == n_mm - 1),
                            )
                            done += 1
                f_sb = fold_halves(ps2, 2)
                tree_epilogue(node_i, f_sb[:], K_MAX)

            def emit_old_node(node_i, start, L):
                nch = L // CHUNK
                vt = oldpath_tiles[start]
                ps2 = apool.tile([K_MAX, 2, F], f32, tag="acc")
                done = 0
                c = 0
                while c < nch:
                    w = 2 if c + 2 <= nch else 1
                    nc.tensor.matmul(
                        ps2[:, 0:w, :], sel_sb[:], vt[:, c : c + w, :],
                        start=(done == 0), stop=(done + w == nch),
                    )
                    done += w
                    c += w
                f_sb = fold_halves(ps2, min(nch, 2))
                tree_epilogue(node_i, f_sb[:], K_MAX)

            def emit_smalls():
                for si, (start, L, depth) in enumerate(small):
                    tree_epilogue(len(big) + si, small_tiles[si][0:L, :], L)

            def emit_local():
                fTl_ps = eppool.tile([F, NLC * CHUNK], f32, tag="fTl_ps", bufs=1)
                for c in range(NLC):
                    nc.tensor.transpose(
                        fTl_ps[:, c * CHUNK : (c + 1) * CHUNK], fl_sb[:, c, :],
                        ident_sb[:],
                    )
                fTl_sb = spool.tile([F, NLC * CHUNK], f32, tag="fTl_sb")
                nc.scalar.copy(fTl_sb[:], fTl_ps[:])
                sl_ps = eppool.tile([HPC, NLC * CHUNK], f32, tag="sl_ps", bufs=1)
                nc.tensor.matmul(sl_ps[:], qbd_sb, fTl_sb[:], start=True, stop=True)
                wl_sb = softmax_weights(sl_ps[:], n_loc, -1)
                for c in range(NLC):
                    wTl_ps = eppool.tile([CHUNK, HPC], f32, tag="qd_ps")
                    nc.tensor.transpose(
                        wTl_ps[:], wl_sb[:, c * CHUNK : (c + 1) * CHUNK],
                        ident_sb[0:HPC, 0:HPC],
                    )
                    wTl_sb = spool.tile([CHUNK, HPC], f32, tag="wTl_sb")
                    nc.scalar.copy(wTl_sb[:], wTl_ps[:])
                    out_matmul(wTl_sb[:], fl_sb[:, c, :])

            # ================= emission schedule =================
            emit_local()
            emit_smalls()
            for i in oldp:
                emit_old_node(i, big[i][0], big[i][1])
            for k, i in enumerate(stream):
                emit_stream_node(i, big[i][0], big[i][1], k == len(stream) - 1)

            # ================= final output =================
            acc_sb = spool.tile([HPC, F], f32, tag="acc_sb")
            nc.scalar.copy(acc_sb[:], out_ps[:])
            # head h's output lives at acc_sb[h, h*64:(h+1)*64]; DMA handles the
            # partition-base-1 read that compute engines can't.
            nc.sync.dma_start(o[0:1, :], acc_sb[0:1, 0:HEAD_DIM])
            nc.sync.dma_start(o[1:2, :], acc_sb[1:2, HEAD_DIM : 2 * HEAD_DIM])

    nc.compile()
    return nc


def _softplus(x):
    return np.log1p(np.exp(-np.abs(x))) + np.maximum(x, 0.0)


def _make_in_maps(v_tokens, q_new, depth_proj_w, depth_temp, pos):
    nodes = cover_set(pos)
    big = [(st, L, d) for (st, L, d) in nodes if L > K_MAX]
    small = [(st, L, d) for (st, L, d) in nodes if L <= K_MAX]
    tree = big + small
    NT = len(tree)
    OFF, CB_W = _cblob_layout(NT)

    selb = np.zeros((CHUNK, SELB_W), np.float32)
    selb[:, 0:K_MAX] = np.tile(np.eye(K_MAX, dtype=np.float32), (CHUNK // K_MAX, 1))
    selb[:, SLIDE0 + 128] = 1.0

    wTI = np.stack(
        [np.eye(HEAD_DIM, dtype=np.float32) + depth_proj_w[d].T for (_, _, d) in tree]
    ) if NT else np.zeros((1, HEAD_DIM, HEAD_DIM), np.float32)
    # folded per-node softmax constants
    rs = np.zeros((max(NT, 1),), np.float32)
    zmul = np.zeros((max(NT, 1),), np.float32)
    for n, (_st, L, d) in enumerate(tree):
        base = 1.0 / ((_softplus(float(depth_temp[d])) + 1e-6) * math.sqrt(HEAD_DIM))
        mean_f = (K_MAX / L) if L > K_MAX else 1.0
        rs[n] = base * mean_f
        zmul[n] = NT / mean_f

    in_maps = []
    for c in range(N_CORES):
        q_c = q_new[0, HPC * c : HPC * (c + 1), :]          # (2, 64)
        cb = np.zeros((CHUNK, CB_W), np.float32)
        cb[:, OFF["ident"] : OFF["ident"] + CHUNK] = np.eye(CHUNK)
        for h in range(HPC):
            cb[h * HEAD_DIM : (h + 1) * HEAD_DIM, OFF["qbd"] + h] = q_c[h]
        cb[0:HEAD_DIM, OFF["qT"] : OFF["qT"] + HPC] = q_c.T
        cb[0:HPC, OFF["rs"] : OFF["rs"] + max(NT, 1)] = rs[None, :]
        cb[0:HPC, OFF["zmul"] : OFF["zmul"] + max(NT, 1)] = zmul[None, :]
        for n in range(max(NT, 1)):
            cb[0:HEAD_DIM, OFF["wTI"] + n * HEAD_DIM : OFF["wTI"] + (n + 1) * HEAD_DIM] = (
                wTI[n] if NT else 0.0
            )
        im = {
            "v": np.ascontiguousarray(
                v_tokens[:, HPC * c : HPC * (c + 1), :]
            ).reshape(NTOK, F),
            "selb": selb,
            "cblob": cb,
        }
        in_maps.append(im)
    return in_maps


def kernel(v_tokens, q_new, depth_proj_w, depth_temp, n_tokens, _profile=False):
    global _last_results
    v_tokens = np.asarray(v_tokens, dtype=np.float32)
    q_new = np.asarray(q_new, dtype=np.float32)
    depth_proj_w = np.asarray(depth_proj_w, dtype=np.float32)
    depth_temp = np.asarray(depth_temp, dtype=np.float32)
    pos = int(n_tokens)

    nc = _build_program(pos, STAGE_A_MODE)
    in_maps = _make_in_maps(v_tokens, q_new, depth_proj_w, depth_temp, pos)
    res = run_bass_kernel_spmd(
        nc, in_maps, core_ids=list(range(N_CORES)), trace=_profile
    )
    _last_results = res

    out = np.zeros((1, NUM_HEADS, HEAD_DIM), np.float32)
    for c in range(N_CORES):
        out[0, HPC * c : HPC * (c + 1), :] = res.results[c]["o"]
    return out


# revision 10
# speedup vs baseline: 1.1868x; 1.1868x over previous
"""Trainium2 Bass kernel for nn_DCWTv2InferenceCache (segment-tree cached attention).

Sharding: tensor-parallel over the 16-head axis -> 8 cores x 2 heads.
Each core streams its (50000, 2*64) f32 slice of the value cache from HBM in
2MB sub-chunks, reduces segment-tree nodes to (64, 128) slot-sums on the PE
(sliding one-hot selection weights, PSUM accumulation), then runs the per-node
depth-projected attention epilogue on-device. Output is head-sharded (2, 64)
per core, gathered on host. No cross-device communication.

Perf structure (vs the first working version):
- the 64 stage-A selection weights are sliding slices B[:, 128-r:192-r] of a
  single (128,192) buffer with one 1.0 per partition -> the old 2.1MB selb
  constant DMA is gone (96KB now).
- the token stream is issued as 2MB sub-chunk DMAs on the sync HWDGE ring
  (constants/prefetches ride the scalar ring), each with its own tile, so the
  PE chases the stream at 2MB granularity instead of 8MB supersteps; the last
  chunks are 1MB to shrink the end-of-stream compute tail.
- softmax scales (softplus temp, mean 64/L, 1/NT) are host-folded; the
  max-subtraction is dropped (logits are O(1) by construction), and the PSUM
  c-halves fold with one DVE add, shortening the tail epilogue chain.
"""

import math
import os
import sys

if "/opt/trn_rl_repo" not in sys.path:
    sys.path.insert(0, "/opt/trn_rl_repo")

import numpy as np

import concourse.bass as bass
import concourse.mybir as mybir
import concourse.tile as tile
from concourse import bacc
from concourse.bass_utils import run_bass_kernel_spmd

# --- problem constants (from the reference nn.Module) ---
MAX_LEN = 65536
NUM_HEADS = 16
HEAD_DIM = 64
K_MAX = 64
LOCAL_WINDOW = 512
LOG_N = 17
LEAF_START = 2**LOG_N

N_CORES = 8
HPC = NUM_HEADS // N_CORES        # heads per core = 2
F = HPC * HEAD_DIM                # feature width per core = 128
NTOK = 50000                      # v_tokens buffer length

CHUNK = 128                       # tokens per matmul tile (partition dim)
SUPER = 16384                     # tokens per stage-A superstep (2 PSUM c-slots)
STAGE_A_MODE = "r64"              # kept for test.py compat

SUBR = int(os.environ.get("DCWT_SUBR", "16"))     # r-slots per stream sub-chunk
TAILR = int(os.environ.get("DCWT_TAILR", "16"))    # r-slots of final sub-chunks
VBUFS = int(os.environ.get("DCWT_VBUFS", "8"))    # stream tile ring depth

SECW = 192                        # one sliding one-hot pattern section
SLIDE0 = 64


def _cblob_layout(NT):
    """Column offsets inside the packed (128, W) f32 constants blob."""
    nt = max(NT, 1)
    off = {}
    off["ident"] = 0
    off["qbd"] = 128
    off["qT"] = 130
    off["rs"] = 132            # (2, NT) folded softmax logit scales
    off["zmul"] = 132 + nt     # (2, NT) folded Z multipliers
    off["wTI"] = 132 + 2 * nt
    return off, 132 + 2 * nt + nt * 64


f32 = mybir.dt.float32
f32r = mybir.dt.float32r
AF = mybir.ActivationFunctionType
AX = mybir.AxisListType

_last_results = None  # stash for test harness introspection


def cover_set(pos):
    """O(log n) segment-tree nodes covering prefix [0..pos-1]: (start, L, depth),
    ascending start (binary decomposition of pos)."""
    if pos <= 0:
        return []
    l, r = LEAF_START, LEAF_START + min(pos, MAX_LEN)
    out = []
    while l < r:
        if l & 1:
            d = LOG_N - int(math.floor(math.log2(l)))
            out.append(((l << d) - LEAF_START, 1 << d, d))
            l += 1
        if r & 1:
            r -= 1
            d = LOG_N - int(math.floor(math.log2(r)))
            out.append(((r << d) - LEAF_START, 1 << d, d))
        l >>= 1
        r >>= 1
    return sorted(out)


def _build_program(pos, mode="r64"):
    """Build the single-core Bass/Tile program (same program for all 8 cores)."""
    nodes = cover_set(pos)
    big = [(s, L, d) for (s, L, d) in nodes if L > K_MAX]
    small = [(s, L, d) for (s, L, d) in nodes if L <= K_MAX]
    tree = big + small                                          # epilogue order
    NT = len(tree)
    stream = [i for i in range(len(big)) if big[i][1] >= SUPER]  # r64 path
    oldp = [i for i in range(len(big)) if big[i][1] < SUPER]     # prefetch path
    n_loc = min(pos, LOCAL_WINDOW)
    assert n_loc % CHUNK == 0, "local window must be chunk-aligned for this build"
    NLC = n_loc // CHUNK

    inv_sqrt_d = 1.0 / math.sqrt(HEAD_DIM)

    nc = bacc.Bacc("TRN2", target_bir_lowering=False, debug=False)

    v = nc.dram_tensor("v", [NTOK, F], f32, kind="ExternalInput")
    CBOFF, CB_W = _cblob_layout(NT)
    CB_IDENT, CB_QBD, CB_QT = CBOFF["ident"], CBOFF["qbd"], CBOFF["qT"]
    CB_RS, CB_ZMUL, CB_WTI = CBOFF["rs"], CBOFF["zmul"], CBOFF["wTI"]
    old_strides = sorted({big[i][1] // CHUNK for i in oldp})
    selb_w = SECW * (1 + len(old_strides))
    selb_d = nc.dram_tensor("selb", [CHUNK, selb_w], f32r, kind="ExternalInput")
    cblob_d = nc.dram_tensor("cblob", [CHUNK, CB_W], f32, kind="ExternalInput")
    o = nc.dram_tensor("o", [HPC, HEAD_DIM], f32, kind="ExternalOutput")

    with tile.TileContext(nc) as tc:
        with (
            tc.tile_pool(name="consts", bufs=1) as cpool,
            tc.tile_pool(name="vstream", bufs=VBUFS) as vpool,
            tc.tile_pool(name="fsb", bufs=2) as fpool,
            tc.tile_pool(name="ep_sb", bufs=2) as spool,
            tc.tile_pool(name="xsb", bufs=3) as xpool,
            tc.tile_pool(name="acc_ps", bufs=2, space=bass.MemorySpace.PSUM) as apool,
            tc.tile_pool(name="ep_ps", bufs=1, space=bass.MemorySpace.PSUM) as eppool,
            tc.tile_pool(name="out_ps", bufs=1, space=bass.MemorySpace.PSUM) as opool,
        ):
            # ---- everything rides the sync HWDGE ring: the scalar ring gets
            # starved by the arbiter behind bulk sync traffic. Two stream
            # chunks go first (engines get a head start), then the small
            # prefetches, then the rest of the stream.
            head_chunks = []

            def emit_head_chunks(n):
                for i in stream:
                    start, L, _d = big[i]
                    for s in range(L // SUPER):
                        base = start + s * SUPER
                        src4 = (
                            v[base : base + SUPER, :]
                            .bitcast(f32r)
                            .rearrange("(c q r) f -> q c r f", c=2, q=CHUNK, r=K_MAX)
                        )
                        r0 = 0
                        while r0 < K_MAX:
                            r1 = min(r0 + SUBR, K_MAX)
                            if len(head_chunks) >= n:
                                return
                            gw = r1 - r0
                            vt = vpool.tile(
                                [CHUNK, 2, gw, F], f32r, tag=f"vb{gw}",
                                bufs=VBUFS if gw == SUBR else 2,
                            )
                            nc.sync.dma_start(vt[:], src4[:, :, r0:r1, :])
                            head_chunks.append(vt)
                            r0 = r1

            emit_head_chunks(2)

            cb = cpool.tile([CHUNK, CB_W], f32)
            nc.sync.dma_start(cb[:], cblob_d[:])
            ident_sb = cb[:, CB_IDENT : CB_IDENT + CHUNK]
            qbd_sb = cb[:, CB_QBD : CB_QBD + HPC]
            qT_sb = cb[0:HEAD_DIM, CB_QT : CB_QT + HPC]
            rs_sb = cb[0:HPC, CB_RS : CB_RS + max(NT, 1)]
            zmul_sb = cb[0:HPC, CB_ZMUL : CB_ZMUL + max(NT, 1)]

            selb_t = cpool.tile([CHUNK, selb_w], f32r)
            nc.sync.dma_start(selb_t[:], selb_d[:])
            selb = selb_t[:]

            def slide_lhsT(r):
                # (128, 64) stream weight: column r all-ones (stride-64 rows)
                return selb[:, SLIDE0 - r : SLIDE0 + 64 - r]

            def old_lhsT(stride, c):
                # (128, 64) weight: partition q -> slot (q*stride + c) % 64
                base = SECW * (1 + old_strides.index(stride))
                return selb[:, base + SLIDE0 - c : base + SLIDE0 + 64 - c]

            def wTI_slice(n):
                return cb[0:HEAD_DIM, CB_WTI + n * HEAD_DIM : CB_WTI + (n + 1) * HEAD_DIM]

            # ---- prefetch raw tail data (small nodes + local window) ----
            small_tiles = []
            for si, (start_s, L_s, _d) in enumerate(small):
                fsm = cpool.tile([K_MAX, F], f32, name=f"fsm{si}", tag=f"fsm{si}")
                nc.sync.dma_start(fsm[0:L_s, :], v[start_s : start_s + L_s, :])
                small_tiles.append(fsm)
            lstart = pos - n_loc
            fl_sb = cpool.tile([CHUNK, NLC, F], f32)
            nc.sync.dma_start(
                fl_sb[:],
                v[lstart : lstart + n_loc, :].rearrange("(q c) f -> q c f", q=CHUNK),
            )

            # old-path (64 < L < SUPER) node prefetch, chunked layout
            oldpath_tiles = {}
            for i in oldp:
                start_b, L_b, _d = big[i]
                nch_b = L_b // CHUNK
                vo = cpool.tile(
                    [CHUNK, nch_b, F], f32r, name=f"vo{start_b}", tag=f"vo{start_b}"
                )
                srcb = v[start_b : start_b + L_b, :].bitcast(f32r)
                nc.sync.dma_start(
                    vo[:], srcb.rearrange("(q c) f -> q c f", q=CHUNK)
                )
                oldpath_tiles[start_b] = vo

            # ---- all tree-node q_depth projections upfront (block-diag) ----
            qd_all = cpool.tile([2 * HEAD_DIM, max(NT, 1), HPC], f32)
            nc.vector.memset(qd_all[:], 0.0)
            for n in range(NT):
                qd_ps = eppool.tile([2 * HEAD_DIM, HPC], f32, tag="qd_ps")
                nc.tensor.matmul(
                    qd_ps[0:HEAD_DIM, 0:1],
                    wTI_slice(n), qT_sb[:, 0:1], start=True, stop=True,
                )
                nc.tensor.matmul(
                    qd_ps[HEAD_DIM : 2 * HEAD_DIM, 1:2],
                    wTI_slice(n), qT_sb[:, 1:2], start=True, stop=True,
                )
                nc.scalar.copy(qd_all[0:HEAD_DIM, n, 0:1], qd_ps[0:HEAD_DIM, 0:1])
                nc.scalar.copy(
                    qd_all[HEAD_DIM : 2 * HEAD_DIM, n, 1:2],
                    qd_ps[HEAD_DIM : 2 * HEAD_DIM, 1:2],
                )

            # ---- cross-node output accumulator (2, 128) PSUM ----
            out_ps = opool.tile([HPC, F], f32)
            n_out_mm = NT + NLC
            out_mm = [0]  # running count, for start/stop flags

            def out_matmul(wT_sb_ap, f_sb_ap):
                nc.tensor.matmul(
                    out_ps[:], wT_sb_ap, f_sb_ap,
                    start=(out_mm[0] == 0), stop=(out_mm[0] == n_out_mm - 1),
                )
                out_mm[0] += 1

            def softmax_weights(s_ps_ap, K, node_i):
                """exp(s*rs_n) / (Z * zmul_n): no max-subtraction (logits are
                O(1) by construction); rs_n folds softplus-temp, sqrt(D) and
                the 64/L mean; zmul_n folds NT and L/64."""
                ebd = xpool.tile([HPC, K], f32, tag="esb")
                zt = xpool.tile([HPC, 1], f32, tag="zt")
                if node_i >= 0:
                    nc.scalar.activation(
                        ebd[:], s_ps_ap, AF.Exp,
                        scale=rs_sb[:, node_i : node_i + 1], accum_out=zt[:],
                    )
                    zs = xpool.tile([HPC, 1], f32, tag="zs")
                    nc.vector.tensor_scalar_mul(
                        zs[:], zt[:], zmul_sb[:, node_i : node_i + 1]
                    )
                    zt = zs
                else:  # local window
                    nc.scalar.activation(
                        ebd[:], s_ps_ap, AF.Exp, scale=inv_sqrt_d, accum_out=zt[:],
                    )
                rz = xpool.tile([HPC, 1], f32, tag="rz")
                nc.vector.reciprocal(rz[:], zt[:])
                w_sb = xpool.tile([HPC, K], f32, tag="wsb")
                nc.vector.tensor_scalar_mul(w_sb[:], ebd[:], rz[:])
                return w_sb

            def tree_epilogue(node_i, f_sb_ap, K):
                """Attention of depth-projected query against f (K, F), one node."""
                fT_ps = eppool.tile([F, K_MAX], f32, tag="fT_ps", bufs=1)
                nc.tensor.transpose(fT_ps[:, 0:K], f_sb_ap, ident_sb[0:K, 0:K])
                fT_sb = spool.tile([F, K_MAX], f32, tag="fT_sb")
                nc.scalar.copy(fT_sb[:, 0:K], fT_ps[:, 0:K])
                s_ps = eppool.tile([HPC, K_MAX], f32, tag="s_ps", bufs=1)
                nc.tensor.matmul(
                    s_ps[:, 0:K], qd_all[:, node_i, :], fT_sb[:, 0:K],
                    start=True, stop=True,
                )
                w_sb = softmax_weights(s_ps[:, 0:K], K, node_i)
                wT_ps = eppool.tile([K_MAX, HPC], f32, tag="qd_ps")
                nc.tensor.transpose(wT_ps[0:K, :], w_sb[:], ident_sb[0:HPC, 0:HPC])
                wT_sb = spool.tile([K_MAX, HPC], f32, tag="wT_sb")
                nc.scalar.copy(wT_sb[0:K, :], wT_ps[0:K, :])
                out_matmul(wT_sb[0:K, :], f_sb_ap)

            def fold_halves(ps2, nhalves):
                """(64, 2, F) PSUM -> (64, F) SBUF slot-sums (scales folded)."""
                f_sb = fpool.tile([K_MAX, F], f32, tag="f")
                if nhalves > 1:
                    # walrus allows only one non-scalar PSUM input per op
                    ha = fpool.tile([K_MAX, F], f32, tag="ha")
                    nc.scalar.copy(ha[:], ps2[:, 0, :])
                    nc.vector.tensor_add(f_sb[:], ha[:], ps2[:, 1, :])
                else:
                    nc.scalar.copy(f_sb[:], ps2[:, 0, :])
                return f_sb

            # ================= streamed stage A (L >= SUPER nodes) ==========
            # Chunk list: (node_i, superstep_base, r0, r1, start?, stop?)
            chunk_no = [0]

            def emit_stream_node(node_i, start, L, last_node):
                SS = L // SUPER
                ps2 = apool.tile([K_MAX, 2, F], f32, tag="acc")
                n_mm = SS * K_MAX
                done = 0
                for s in range(SS):
                    base = start + s * SUPER
                    src4 = (
                        v[base : base + SUPER, :]
                        .bitcast(f32r)
                        .rearrange("(c q r) f -> q c r f", c=2, q=CHUNK, r=K_MAX)
                    )
                    r0 = 0
                    while r0 < K_MAX:
                        r1 = min(r0 + SUBR, K_MAX)
                        gw = r1 - r0
                        if chunk_no[0] < len(head_chunks):
                            vt = head_chunks[chunk_no[0]]
                        else:
                            vt = vpool.tile(
                                [CHUNK, 2, gw, F], f32r, tag=f"vb{gw}",
                                bufs=VBUFS if gw == SUBR else 2,
                            )
                            nc.sync.dma_start(vt[:], src4[:, :, r0:r1, :])
                        chunk_no[0] += 1
                        for rr in range(r0, r1):
                            nc.tensor.matmul(
                                ps2[:, :, :], slide_lhsT(rr), vt[:, :, rr - r0, :],
                                start=(done == 0), stop=(done == n_mm - 1),
                            )
                            done += 1
                        r0 = r1
                f_sb = fold_halves(ps2, 2)
                tree_epilogue(node_i, f_sb[:], K_MAX)

            def emit_old_node(node_i, start, L):
                nch = L // CHUNK
                vt = oldpath_tiles[start]
                ps2 = apool.tile([K_MAX, 2, F], f32, tag="acc")
                for c in range(nch):
                    nc.tensor.matmul(
                        ps2[:, 0, :], old_lhsT(nch, c), vt[:, c, :],
                        start=(c == 0), stop=(c == nch - 1),
                    )
                f_sb = fold_halves(ps2, 1)
                tree_epilogue(node_i, f_sb[:], K_MAX)

            def emit_smalls():
                for si, (start, L, depth) in enumerate(small):
                    tree_epilogue(len(big) + si, small_tiles[si][0:L, :], L)

            def emit_local():
                fTl_ps = eppool.tile([F, NLC * CHUNK], f32, tag="fTl_ps", bufs=1)
                for c in range(NLC):
                    nc.tensor.transpose(
                        fTl_ps[:, c * CHUNK : (c + 1) * CHUNK], fl_sb[:, c, :],
                        ident_sb[:],
                    )
                fTl_sb = spool.tile([F, NLC * CHUNK], f32, tag="fTl_sb")
                nc.scalar.copy(fTl_sb[:], fTl_ps[:])
                sl_ps = eppool.tile([HPC, NLC * CHUNK], f32, tag="sl_ps", bufs=1)
                nc.tensor.matmul(sl_ps[:], qbd_sb, fTl_sb[:], start=True, stop=True)
                wl_sb = softmax_weights(sl_ps[:], n_loc, -1)
                for c in range(NLC):
                    wTl_ps = eppool.tile([CHUNK, HPC], f32, tag="qd_ps")
                    nc.tensor.transpose(
                        wTl_ps[:], wl_sb[:, c * CHUNK : (c + 1) * CHUNK],
                        ident_sb[0:HPC, 0:HPC],
                    )
                    wTl_sb = spool.tile([CHUNK, HPC], f32, tag="wTl_sb")
                    nc.scalar.copy(wTl_sb[:], wTl_ps[:])
                    out_matmul(wTl_sb[:], fl_sb[:, c, :])

            # ================= emission schedule =================
            emit_local()
            emit_smalls()
            for i in oldp:
                emit_old_node(i, big[i][0], big[i][1])
            for k, i in enumerate(stream):
                emit_stream_node(i, big[i][0], big[i][1], k == len(stream) - 1)

            # ================= final output =================
            acc_sb = spool.tile([HPC, F], f32, tag="acc_sb")
            nc.scalar.copy(acc_sb[:], out_ps[:])
            # head h's output lives at acc_sb[h, h*64:(h+1)*64]; DMA handles the
            # partition-base-1 read that compute engines can't.
            nc.sync.dma_start(o[0:1, :], acc_sb[0:1, 0:HEAD_DIM])
            nc.sync.dma_start(o[1:2, :], acc_sb[1:2, HEAD_DIM : 2 * HEAD_DIM])

    nc.compile()
    return nc


def _softplus(x):
    return np.log1p(np.exp(-np.abs(x))) + np.maximum(x, 0.0)


def _make_in_maps(v_tokens, q_new, depth_proj_w, depth_temp, pos):
    nodes = cover_set(pos)
    big = [(st, L, d) for (st, L, d) in nodes if L > K_MAX]
    small = [(st, L, d) for (st, L, d) in nodes if L <= K_MAX]
    tree = big + small
    NT = len(tree)
    OFF, CB_W = _cblob_layout(NT)

    old_strides = sorted({L // CHUNK for (_s, L, _d) in big if L < SUPER})
    selb = np.zeros((CHUNK, SECW * (1 + len(old_strides))), np.float32)
    selb[:, SLIDE0] = 1.0                               # stream pattern (stride 64)
    for i, s in enumerate(old_strides):
        base = SECW * (1 + i)
        q = np.arange(CHUNK)
        selb[q, base + SLIDE0 + K_MAX + (q * s) % K_MAX - K_MAX] = 1.0

    wTI = np.stack(
        [np.eye(HEAD_DIM, dtype=np.float32) + depth_proj_w[d].T for (_, _, d) in tree]
    ) if NT else np.zeros((1, HEAD_DIM, HEAD_DIM), np.float32)
    # folded per-node softmax constants
    rs = np.zeros((max(NT, 1),), np.float32)
    zmul = np.zeros((max(NT, 1),), np.float32)
    for n, (_st, L, d) in enumerate(tree):
        base = 1.0 / ((_softplus(float(depth_temp[d])) + 1e-6) * math.sqrt(HEAD_DIM))
        mean_f = (K_MAX / L) if L > K_MAX else 1.0
        rs[n] = base * mean_f
        zmul[n] = NT / mean_f

    in_maps = []
    for c in range(N_CORES):
        q_c = q_new[0, HPC * c : HPC * (c + 1), :]          # (2, 64)
        cb = np.zeros((CHUNK, CB_W), np.float32)
        cb[:, OFF["ident"] : OFF["ident"] + CHUNK] = np.eye(CHUNK)
        for h in range(HPC):
            cb[h * HEAD_DIM : (h + 1) * HEAD_DIM, OFF["qbd"] + h] = q_c[h]
        cb[0:HEAD_DIM, OFF["qT"] : OFF["qT"] + HPC] = q_c.T
        cb[0:HPC, OFF["rs"] : OFF["rs"] + max(NT, 1)] = rs[None, :]
        cb[0:HPC, OFF["zmul"] : OFF["zmul"] + max(NT, 1)] = zmul[None, :]
        for n in range(max(NT, 1)):
            cb[0:HEAD_DIM, OFF["wTI"] + n * HEAD_DIM : OFF["wTI"] + (n + 1) * HEAD_DIM] = (
                wTI[n] if NT else 0.0
            )
        im = {
            "v": np.ascontiguousarray(
                v_tokens[:, HPC * c : HPC * (c + 1), :]
            ).reshape(NTOK, F),
            "selb": selb,
            "cblob": cb,
        }
        in_maps.append(im)
    return in_maps


def kernel(v_tokens, q_new, depth_proj_w, depth_temp, n_tokens, _profile=False):
    global _last_results
    v_tokens = np.asarray(v_tokens, dtype=np.float32)
    q_new = np.asarray(q_new, dtype=np.float32)
    depth_proj_w = np.asarray(depth_proj_w, dtype=np.float32)
    depth_temp = np.asarray(depth_temp, dtype=np.float32)
    pos = int(n_tokens)

    nc = _build_program(pos, STAGE_A_MODE)
    in_maps = _make_in_maps(v_tokens, q_new, depth_proj_w, depth_temp, pos)
    res = run_bass_kernel_spmd(
        nc, in_maps, core_ids=list(range(N_CORES)), trace=_profile
    )
    _last_results = res

    out = np.zeros((1, NUM_HEADS, HEAD_DIM), np.float32)
    for c in range(N_CORES):
        out[0, HPC * c : HPC * (c + 1), :] = res.results[c]["o"]
    return out
